# revision 9
# baseline (speedup 1.0000x reference)
"""Trainium2 Bass kernel for ComiRec dynamic-routing (CapsNet-style) layer.

Problem: B=1024, S=200, E=128, C=128, n_caps=4, 3 routing rounds.

Sharding (8 cores): core i handles capsule n = i//2 and batch half h = i%2
(512 batch rows). Capsules are fully independent in the reference math, so
there is no cross-core communication. Each core processes its 512 batch rows
in 4 chunks of 128 (the partition width).

Per-core dataflow (per 128-row chunk):
  stage A : u[b,s,c] = (mask*behaviors)[b,s,:] @ W[n,s]  (PE matmuls, bf16,
            fp32 PSUM). Each [b,4s,C] PSUM bank is evacuated TWICE: into
            u_sc (two [b,100,C] s-half tiles) and into u_cs (two [b,C,100]
            s-major tiles, strided dst), both on the scalar engine (chunk 0
            splits the work with the then-idle vector engine). A second
            accumulating matmul chain computes z0_raw[b,c] = sum_s u.
  round 0 : caps0 = squash(z0_raw / n_valid); logits = delta0 + mneg.
  rounds  : coup = exp(logits) in bf16 with f32 accum (no max subtraction:
            valid logits are bounded by ~13, masked ones are -3e38 -> 0).
            caps_pre[b,c] = sum_s coup*u : broadcast TT mult on u_cs halves
            (coup broadcast over middle c axis -> DVE 2x mode) + pair-tree
            reduce over s.  delta[b,s] = sum_c u*caps : broadcast TT mult
            on u_sc halves + pair-tree reduce over c.
  squash  : alpha = sqrt(n2)/(1+n2) via exp(0.5*ln(n2) - ln(1+n2)); the two
            ln's are batched into one ACT call; n2 comes from a DVE
            multiply-accumulate, so ACT stays on the exp/ln table set for
            the whole kernel.

The kernel() entry takes FULL inputs (as produced by the problem's
setup_inputs) and returns the FULL [1024, 4, 128] fp32 output.
"""

import numpy as np
import ml_dtypes

BF16 = ml_dtypes.bfloat16

B, S, E, C, NCAPS = 1024, 200, 128, 128, 4
NCORES = 8
BH = B // 2          # batch rows per core (one half)
P = 128              # partition width / chunk size
NCHUNK = BH // P     # 4 chunks per core
SBLK = 8             # s-tiles per DMA block
SH = S // 2          # s-half length (100)
NEG = -3.0e38

_COMPILED = {}


def _emit(ctx, tc, nc):
    import concourse.bass as bass
    from concourse import mybir

    bf = mybir.dt.bfloat16
    f32 = mybir.dt.float32

    nblk = S // SBLK
    bmt = nc.dram_tensor(
        "bmt", [NCHUNK, nblk, E, SBLK, P], bf, kind="ExternalInput").ap()
    w = nc.dram_tensor(
        "w", [nblk, E, SBLK, C], bf, kind="ExternalInput").ap()
    mneg = nc.dram_tensor("mneg", [NCHUNK, P, S], f32, kind="ExternalInput").ap()
    invn = nc.dram_tensor("invn", [NCHUNK, P, 1], f32, kind="ExternalInput").ap()
    out = nc.dram_tensor("caps_out", [NCHUNK, P, C], f32, kind="ExternalOutput").ap()

    wpool = ctx.enter_context(tc.tile_pool(name="wstream", bufs=3))
    bmtpool = ctx.enter_context(tc.tile_pool(name="bmt", bufs=3))
    uscpool = ctx.enter_context(tc.tile_pool(name="usc", bufs=3))
    ucspool = ctx.enter_context(tc.tile_pool(name="ucs", bufs=3))
    scrpool = ctx.enter_context(tc.tile_pool(name="scr", bufs=1))
    smalls = ctx.enter_context(tc.tile_pool(name="smalls", bufs=2))
    pupool = ctx.enter_context(tc.tile_pool(name="pu", bufs=6, space="PSUM"))
    pzpool = ctx.enter_context(tc.tile_pool(name="pz", bufs=2, space="PSUM"))

    def tree_reduce_inner(src, width, dst):
        # src: [P, outer, width] bf16; pairwise adds over the inner axis,
        # in place; final level writes dst [P, outer] f32.
        wlen = width
        while wlen > 2:
            half = wlen // 2
            nxt = (wlen + 1) // 2
            nc.vector.tensor_add(
                src[:, :, 0:half], src[:, :, 0:half], src[:, :, nxt:nxt + half]
            )
            wlen = nxt
        nc.vector.tensor_add(dst, src[:, :, 0], src[:, :, 1])

    def squash_scale(zraw, invz, capsb, caps=None):
        # capsb = zraw * alpha * invz (bf16), alpha = sqrt(n2)/(1+n2),
        # n2 = |zraw*invz|^2.  rsqrt(n2) via the fast-inverse-sqrt bit
        # trick + one Newton step, all on the vector engine so the scalar
        # engine never leaves the exp table set.
        u32 = mybir.dt.uint32
        n2 = smalls.tile([P, 1], f32, tag="n2")
        sqt = smalls.tile([P, C], f32, tag="sqt")
        nc.vector.scalar_tensor_tensor(
            out=sqt, in0=zraw, scalar=1.0, in1=zraw,
            op0=mybir.AluOpType.mult, op1=mybir.AluOpType.mult,
            accum_out=n2,
        )
        x = smalls.tile([P, 1], f32, tag="x")
        nc.vector.tensor_scalar(
            out=x, in0=n2, scalar1=invz, scalar2=invz,
            op0=mybir.AluOpType.mult, op1=mybir.AluOpType.mult,
        )
        # seed: y = bits(0x5f3759df - (bits(x) >> 1)).  DVE uint add
        # saturates, so the subtraction runs in float domain (seed bit
        # error ~64 ulp, fixed by the Newton steps).
        y = smalls.tile([P, 1], f32, tag="y")
        t0 = smalls.tile([P, 1], f32, tag="t0")
        nc.vector.tensor_scalar(
            out=t0.bitcast(u32), in0=x.bitcast(u32),
            scalar1=1, scalar2=None,
            op0=mybir.AluOpType.logical_shift_right,
        )
        nc.vector.tensor_copy(t0, t0.bitcast(u32))
        nc.vector.tensor_scalar(
            out=t0, in0=t0, scalar1=-1.0, scalar2=float(0x5f3759df),
            op0=mybir.AluOpType.mult, op1=mybir.AluOpType.add,
        )
        nc.vector.tensor_copy(y.bitcast(u32), t0)
        # Newton: y *= 1.5 - 0.5*x*y*y
        t1 = smalls.tile([P, 1], f32, tag="t1")
        nc.vector.tensor_mul(t1, y, y)
        nc.vector.tensor_mul(t1, t1, x)
        nc.vector.tensor_scalar(
            out=t1, in0=t1, scalar1=-0.5, scalar2=1.5,
            op0=mybir.AluOpType.mult, op1=mybir.AluOpType.add,
        )
        nc.vector.tensor_mul(y, y, t1)
        # second Newton step (cheap, keeps alpha error ~1e-5)
        nc.vector.tensor_mul(t1, y, y)
        nc.vector.tensor_mul(t1, t1, x)
        nc.vector.tensor_scalar(
            out=t1, in0=t1, scalar1=-0.5, scalar2=1.5,
            op0=mybir.AluOpType.mult, op1=mybir.AluOpType.add,
        )
        nc.vector.tensor_mul(y, y, t1)
        # alpha = x*y/(1+x);  capsb = zraw * alpha * invz
        num = smalls.tile([P, 1], f32, tag="num")
        nc.vector.tensor_mul(num, x, y)
        den = smalls.tile([P, 1], f32, tag="den")
        nc.vector.tensor_scalar_add(den, x, 1.0)
        dinv = smalls.tile([P, 1], f32, tag="dinv")
        nc.vector.reciprocal(dinv, den)
        alpha = smalls.tile([P, 1], f32, tag="alpha")
        nc.vector.tensor_mul(alpha, num, dinv)
        nc.vector.tensor_scalar(
            out=capsb, in0=zraw, scalar1=alpha, scalar2=invz,
            op0=mybir.AluOpType.mult, op1=mybir.AluOpType.mult,
        )
        if caps is not None:
            nc.vector.tensor_scalar(
                out=caps, in0=zraw, scalar1=alpha, scalar2=invz,
                op0=mybir.AluOpType.mult, op1=mybir.AluOpType.mult,
            )

    for k in range(NCHUNK):
        mneg_sb = smalls.tile([P, S], f32, tag="mneg")
        nc.sync.dma_start(out=mneg_sb, in_=mneg[k])
        invn_sb = smalls.tile([P, 1], f32, tag="invn")
        nc.sync.dma_start(out=invn_sb, in_=invn[k])

        usc = [uscpool.tile([P, SH, C], bf, tag="usc", name=f"usc{hh}")
               for hh in range(2)]
        ucs = [ucspool.tile([P, C, SH], bf, tag="ucs", name=f"ucs{hh}")
               for hh in range(2)]
        scr = scrpool.tile([P, SH * C], bf, tag="scr")
        scr_sc = scr.rearrange("p (a c) -> p a c", a=SH)   # [P, 100, C]
        scr_cs = scr.rearrange("p (c a) -> p c a", c=C)    # [P, C, 100]
        pz = pzpool.tile([P, C], f32, tag="pz")

        # ---------- stage A ----------
        pu = None
        for blk in range(S // SBLK):
            bt = bmtpool.tile([E, SBLK, P], bf, tag="bt")
            wt_blk = wpool.tile([E, SBLK, C], bf, tag="wt")
            nc.sync.dma_start(out=bt, in_=bmt[k, blk])
            nc.sync.dma_start(out=wt_blk, in_=w[blk])
            for j in range(SBLK):
                s = blk * SBLK + j
                q = s % 4
                if q == 0:
                    pu = pupool.tile([P, 4 * C], f32, tag="pu")
                nc.tensor.matmul(
                    pu[:, q * C:(q + 1) * C], lhsT=bt[:, j, :], rhs=wt_blk[:, j, :],
                    start=True, stop=True,
                )
                nc.tensor.matmul(
                    pz, lhsT=bt[:, j, :], rhs=wt_blk[:, j, :],
                    start=(s == 0), stop=(s == S - 1), skip_group_check=True,
                )
                if q == 3:
                    pv = pu.rearrange("p (a c) -> p a c", c=C)
                    h = s // SH
                    srel = (s - 3) % SH
                    # chunk 0: the vector engine is idle during the first
                    # stage A, so it takes half the evacuation copies.
                    if k == 0 and (s // 4) % 2 == 0:
                        nc.vector.tensor_copy(usc[h][:, srel:srel + 4, :], pv)
                    else:
                        nc.scalar.copy(usc[h][:, srel:srel + 4, :], pv)
                    nc.scalar.copy(
                        ucs[h][:, :, srel:srel + 4],
                        pv.rearrange("p a c -> p c a"),
                    )

        logits = smalls.tile([P, S], f32, tag="logits")
        caps = smalls.tile([P, C], f32, tag="caps")
        capsb = smalls.tile([P, C], bf, tag="capsb")
        zraw = smalls.tile([P, C], f32, tag="zraw")
        invz = smalls.tile([P, 1], f32, tag="invz")

        def delta_into_logits(r):
            # r==0: logits = delta + mneg ; r==1: logits += delta
            for h in range(2):
                capse = capsb.unsqueeze(1).broadcast_to([P, SH, C])
                nc.vector.tensor_mul(scr_sc, usc[h], capse)
                lsl = logits[:, h * SH:(h + 1) * SH]
                dtmp = smalls.tile([P, SH], f32, tag="dtmp")
                tree_reduce_inner(scr_sc, C, dtmp)
                if r == 0:
                    nc.vector.tensor_add(
                        lsl, dtmp, mneg_sb[:, h * SH:(h + 1) * SH])
                else:
                    nc.vector.tensor_add(lsl, lsl, dtmp)

        for r in range(3):
            if r == 0:
                nc.scalar.copy(zraw, pz)
                squash_scale(zraw, invn_sb, capsb)
            else:
                # coup = exp(logits); masked entries are -3e38 -> exp -> 0
                coupb = smalls.tile([P, S], bf, tag="coupb")
                zsum = smalls.tile([P, 1], f32, tag="zsum")
                nc.scalar.activation(
                    out=coupb, in_=logits, func=mybir.ActivationFunctionType.Exp,
                    accum_out=zsum,
                )
                nc.vector.reciprocal(invz, zsum)
                # caps_pre = sum_s coup * u, via u_cs halves
                zr = [smalls.tile([P, C], f32, tag=f"zr{hh}", name=f"zr{hh}")
                      for hh in range(2)]
                for h in range(2):
                    coupe = (
                        coupb[:, h * SH:(h + 1) * SH]
                        .unsqueeze(1).broadcast_to([P, C, SH])
                    )
                    nc.vector.tensor_mul(scr_cs, ucs[h], coupe)
                    tree_reduce_inner(scr_cs, SH, zr[h])
                nc.vector.tensor_add(zraw, zr[0], zr[1])
                squash_scale(zraw, invz, capsb, caps=(caps if r == 2 else None))

            if r < 2:
                delta_into_logits(r)

        nc.sync.dma_start(out=out[k], in_=caps)


def _build():
    if "nc" in _COMPILED:
        return _COMPILED["nc"]
    from contextlib import ExitStack
    import concourse.bacc as bacc
    import concourse.tile as tile

    nc = bacc.Bacc(
        "TRN2", target_bir_lowering=False, debug=False, enable_asserts=False
    )
    with tile.TileContext(nc, trace_sim=False) as tc, ExitStack() as ctx:
        _emit(ctx, tc, nc)
    nc.compile()
    _COMPILED["nc"] = nc
    return nc


def make_in_maps(behaviors, valid_mask, W):
    behaviors = np.asarray(behaviors, dtype=np.float32)
    mask = np.asarray(valid_mask).astype(bool)
    W = np.asarray(W, dtype=np.float32)

    nblk = S // SBLK
    bm = behaviors * mask[:, :, None].astype(np.float32)
    bmt_full = np.ascontiguousarray(bm.transpose(1, 2, 0)).astype(BF16)  # [S,E,B]
    w_bf = W.astype(BF16)                                               # [N,S,E,C]
    mneg_full = np.where(mask, 0.0, NEG).astype(np.float32)             # [B,S]
    nval = mask.sum(axis=1).astype(np.float32)
    invn_full = (1.0 / np.maximum(nval, 1.0)).astype(np.float32)        # [B]

    in_maps = []
    for core in range(NCORES):
        n, h = core // 2, core % 2
        bsl = slice(h * BH, (h + 1) * BH)
        bmt_core = bmt_full[:, :, bsl]                  # [S, E, BH]
        bmt_blk = np.ascontiguousarray(
            bmt_core.reshape(nblk, SBLK, E, NCHUNK, P)
            .transpose(3, 0, 2, 1, 4))                      # [NCHUNK,nblk,E,SBLK,P]
        w_blk = np.ascontiguousarray(
            w_bf[n].reshape(nblk, SBLK, E, C)
            .transpose(0, 2, 1, 3))                         # [nblk,E,SBLK,C]
        in_maps.append({
            "bmt": bmt_blk,
            "w": w_blk,
            "mneg": np.ascontiguousarray(
                mneg_full[bsl].reshape(NCHUNK, P, S)),
            "invn": np.ascontiguousarray(
                invn_full[bsl].reshape(NCHUNK, P, 1)),
        })
    return in_maps


def gather_output(results):
    out = np.empty((B, NCAPS, C), dtype=np.float32)
    for core in range(NCORES):
        n, h = core // 2, core % 2
        caps = results[core]["caps_out"].reshape(BH, C)
        out[h * BH:(h + 1) * BH, n, :] = caps
    return out


def kernel(behaviors, valid_mask, W):
    from concourse import bass_utils

    nc = _build()
    in_maps = make_in_maps(behaviors, valid_mask, W)
    res = bass_utils.run_bass_kernel_spmd(nc, in_maps, core_ids=list(range(NCORES)))
    return gather_output(res.results)
